# revision 70
# baseline (speedup 1.0000x reference)
"""GQA attention kernel for 8 trn2 NeuronCores (tensor-parallel over heads).

Problem: B=1, S=2048, D=2048, NQ=32 q heads, NKV=8 kv heads, HD=64.
Core i handles q heads 4i..4i+3 and kv head i; out = sum of per-core partials.

v2: all matmuls in bf16 (fp32 runs at 1/4 rate on the PE), x^T prepared on
host (kills 256 on-device PE transposes), paired-head score matmuls issued
to disjoint PE row groups (K=64 each -> concurrent), exp batched over
[128, 2x512] PSUM groups, bf16 partial outputs summed on host.

Layout (all transposed, zero on-device transposes):
  xT  [128, 4(sc), 16(kc), 512] bf16   host-pretransposed activations
  Q^T [128=2 heads x 64, S] per head pair, K^T [128, S] (dup for row pairing)
  V   [S, 16, 64+1] bf16 + ones column (softmax sums come free from PV)
  S^T block pair = KT.T @ QT (two row-group matmuls) -> one exp on ACT
  PV: V_ext.T @ expS^T ; row 64 = softmax denominators
  out-proj: lhsT = O^T directly, bf16 partial written to DRAM
RMSNorm over head dim (= partitions) via ones-selector matmuls on PE.
"""

import os
import sys

sys.path.insert(0, "/opt/trn_rl_repo")

import numpy as np
import ml_dtypes

BF16 = ml_dtypes.bfloat16

S = 2048
D = 2048
HD = 64
NQ = 32
NKV = 8
P = 128
EPS = 1e-6
SCALE = 0.125  # 1/sqrt(HD)
N_CORES = 8

_CACHE = {}
LAST_RESULTS = None


def _build_nc():
    import concourse.bass as bass
    import concourse.tile as tile
    from concourse import bacc, mybir

    f32 = mybir.dt.float32
    bf = mybir.dt.bfloat16
    nc = bacc.Bacc("TRN2", target_bir_lowering=False, debug=False)

    def dram_in(name, shape, dt):
        return nc.dram_tensor(name, list(shape), dt, kind="ExternalInput").ap()

    io = {
        "xt4": dram_in("xt4", (P, 4, 16, 512), bf),
        "wqa": dram_in("wqa", (P, 16, P), bf),
        "wqb": dram_in("wqb", (P, 16, P), bf),
        "wk": dram_in("wk", (P, 16, HD), bf),
        "wv": dram_in("wv", (P, 16, HD), bf),
        "wo": dram_in("wo", (P, 2, D), bf),
        "cos4": dram_in("cos4", (P, S), bf),
        "sin4s": dram_in("sin4s", (P, S), bf),
        "gq2": dram_in("gq2", (P, 1), f32),
        "gk": dram_in("gk", (HD, 1), f32),
        "masktri": dram_in("masktri", (P, 2, P), bf),
        "ones65": dram_in("ones65", (P, HD + 1), bf),
        "rot2": dram_in("rot2", (P, P), bf),
        "out": nc.dram_tensor("out", [S, D], bf, kind="ExternalOutput").ap(),
    }

    from contextlib import ExitStack

    with tile.TileContext(nc) as tc, ExitStack() as ctx:
        _emit(ctx, tc, io, bass, mybir)
    nc.compile()
    return nc


def _emit(ctx, tc, io, bass, mybir):
    nc = tc.nc
    f32 = mybir.dt.float32
    bf = mybir.dt.bfloat16
    Exp = mybir.ActivationFunctionType.Exp
    Sqrt = mybir.ActivationFunctionType.Sqrt
    Square = mybir.ActivationFunctionType.Square
    Copy = mybir.ActivationFunctionType.Copy
    mult = mybir.AluOpType.mult
    add = mybir.AluOpType.add

    cpool = ctx.enter_context(tc.tile_pool(name="consts", bufs=1))
    pers = ctx.enter_context(tc.tile_pool(name="persist", bufs=1))

    # ---- inputs: first seq chunk + first-needed weights lead the DMA queue
    xip = ctx.enter_context(tc.tile_pool(name="xin", bufs=3))
    xs_t = [None] * 4

    def load_xs(sc):
        xs_t[sc] = xip.tile([P, 16, 512], bf, tag="xs", name="xs")
        nc.sync.dma_start(xs_t[sc][:], io["xt4"][:, sc, :, :])

    def cload(name, shape, dt):
        t = cpool.tile(list(shape), dt, tag=name, name=name)
        nc.sync.dma_start(t[:], io[name][:])
        return t

    load_xs(0)
    wqa = cload("wqa", (P, 16, P), bf)
    wk = cload("wk", (P, 16, HD), bf)
    wv = cload("wv", (P, 16, HD), bf)
    wqb = cload("wqb", (P, 16, P), bf)
    gq2 = cload("gq2", (P, 1), f32)
    gk = cload("gk", (HD, 1), f32)
    cos4 = cload("cos4", (P, S), bf)
    sin4s = cload("sin4s", (P, S), bf)
    rot2 = cload("rot2", (P, P), bf)
    ones65 = cload("ones65", (P, HD + 1), bf)
    load_xs(1)
    masktri = cload("masktri", (P, 2, P), bf)
    wo = cload("wo", (P, 2, D), bf)

    # ---- persistent activations ----
    QT = [pers.tile([P, S], bf, tag=f"qt{t}", name=f"QT{t}") for t in range(2)]
    KT = pers.tile([P, S], bf, tag="kt")  # rows 64-127 = duplicate of 0-63
    V = pers.tile([P, 16, HD + 1], bf, tag="v")  # [seq128, kblock, hd+ones]
    OT = pers.tile([P, 2, S], bf, tag="ot")  # attn out transposed

    nc.vector.memset(V[:, :, HD : HD + 1], 1.0)
    epsc = pers.tile([P, 1], f32, tag="epsc")
    nc.vector.memset(epsc[:], EPS)
    jnk = pers.tile([P, P], bf, tag="jnk")
    nc.vector.memset(jnk[:], 0.0)
    # one exp-output slot per key block: fresh region per kb, so the ACT
    # queue carries no buffer-reuse WAR waits inside an attention chunk
    ering = pers.tile([P, 16, 2, 512], bf, tag="ering")

    # ============ Phase 1+2: projections + RMSNorm + RoPE ============
    with (
        tc.tile_pool(name="sq", bufs=2) as sqp,
        tc.tile_pool(name="stdv", bufs=2) as stdp,
        tc.tile_pool(name="rstd", bufs=2) as rsp,
        tc.tile_pool(name="bcast", bufs=2) as bcp,
        tc.tile_pool(name="tnorm", bufs=2) as tnp,
        tc.tile_pool(name="tcos", bufs=2) as tcp,
        tc.tile_pool(name="tsin", bufs=2) as t1p,
        tc.tile_pool(name="ppsum", bufs=3, space="PSUM") as pp,
        tc.tile_pool(name="sspsum", bufs=1, space="PSUM") as ssp,
        tc.tile_pool(name="swpsum", bufs=2, space="PSUM") as swp,
        tc.tile_pool(name="vpsum", bufs=2, space="PSUM") as vp,
    ):
        # warm-up matmuls (no DMA dependency) so the PE clock is already
        # un-throttled when the first projection lands
        for _ in range(96):
            jp = pp.tile([P, 512], f32, tag="p", name="jp")
            nc.tensor.matmul(jp[:, 0:HD], jnk[:], jnk[:, 0:HD], start=True,
                             stop=True)
        for sc in range(4):
            cs = slice(sc * 512, (sc + 1) * 512)
            if sc < 2:
                load_xs(sc + 2)
            xs = xs_t[sc]

            def proj_norm_rope(lhsT_w, m, g, sel, nh, dst):
                # projection into PSUM
                ps = pp.tile([P, 512], f32, tag="p", name="ps")[:m]
                for kc in range(16):
                    nc.tensor.matmul(
                        ps, lhsT_w[:, kc, :], xs[:, kc, :],
                        start=(kc == 0), stop=(kc == 15),
                    )
                # rms stats: sq = ps^2 (bf16); head sums land at partitions
                # 0 and 64 (selector cols 0/64) so all slices are p0/p64
                sq = sqp.tile([P, 512], bf, tag="sq", name="sq")[:m]
                nc.scalar.activation(sq, ps, Square)
                nss = HD + 1 if nh == 2 else 1
                ssps = ssp.tile([HD + 1, 512], f32, tag="ss", name="ssps")[:nss]
                nc.tensor.matmul(ssps, sel[:m, :nss], sq, start=True, stop=True)
                std = stdp.tile([HD + 1, 512], f32, tag="std", name="std")[:nss]
                nc.scalar.activation(std, ssps, Sqrt, bias=epsc[:nss],
                                     scale=1.0 / HD)
                # gpsimd broadcast can only write partition-0-based tiles;
                # the upper head group goes via a staging tile + SBUF DMA
                # approx_fast and partition_broadcast both need partition-0
                # sources on HW; relocate the p64 row via a DVE copy first
                bc = bcp.tile([P, 512], f32, tag="bc", name="bc")[:m]
                for h in range(nh):
                    rstd = rsp.tile([1, 512], f32, tag=f"rstd{h}",
                                    name=f"rstd{h}")
                    if h == 0:
                        nc.vector.reciprocal_approx_fast(rstd, std[0:1, :])
                        nc.gpsimd.partition_broadcast(bc[0:HD, :], rstd)
                    else:
                        stdc = rsp.tile([1, 512], f32, tag="stdc", name="stdc")
                        nc.vector.tensor_copy(stdc, std[HD : HD + 1, :])
                        nc.vector.reciprocal_approx_fast(rstd, stdc)
                        bch = bcp.tile([HD, 512], f32, tag="bch", name="bch")
                        nc.gpsimd.partition_broadcast(bch, rstd)
                        nc.sync.dma_start(bc[HD:P, :], bch[:])
                # normalize: tn = (ps * g) * bc   (bf16 out)
                tn = tnp.tile([P, 512], bf, tag="tn", name="tn")[:m]
                nc.vector.scalar_tensor_tensor(tn, ps, g, bc, mult, mult)
                # rope: dst = tn*cos + swap(tn)*sin
                tmpc = tcp.tile([P, 512], bf, tag="tc", name="tmpc")[:m]
                nc.vector.tensor_mul(tmpc, tn, cos4[:m, cs])
                sw = swp.tile([P, 512], f32, tag="sw", name="sw")[:m]
                nc.tensor.matmul(sw, rot2[:m, :m], tn, start=True, stop=True)
                t1 = t1p.tile([P, 512], bf, tag="t1", name="t1")[:m]
                nc.vector.tensor_mul(t1, sw, sin4s[:m, cs])
                nc.vector.tensor_add(dst, t1, tmpc)

            proj_norm_rope(wqa, P, gq2[:, :], ones65, 2, QT[0][:, cs])
            proj_norm_rope(wqb, P, gq2[:, :], ones65, 2, QT[1][:, cs])
            proj_norm_rope(wk, HD, gk[:, :], ones65, 1, KT[0:HD, cs])
            # duplicate normed+roped K into partitions 64-127 (row pairing)
            nc.sync.dma_start(KT[HD:P, cs], KT[0:HD, cs])
            # V in normal layout: lhsT = x^T slice, rhs = wv
            for ms in range(4):
                pv = vp.tile([P, HD], f32, tag="v", name="pv")
                for kc in range(16):
                    nc.tensor.matmul(
                        pv[:], xs[:, kc, ms * P : (ms + 1) * P], wv[:, kc, :],
                        start=(kc == 0), stop=(kc == 15),
                    )
                nc.vector.tensor_copy(V[:, sc * 4 + ms, 0:HD], pv[:])

    # ============ Phase 3: attention + out-projection ============
    with (
        tc.tile_pool(name="exps", bufs=4) as ep,
        tc.tile_pool(name="recs", bufs=2) as rcp,
        tc.tile_pool(name="bcs", bufs=2) as bcsp,
        tc.tile_pool(name="stg", bufs=2) as stgp,
        tc.tile_pool(name="ov", bufs=2) as ovp,
        tc.tile_pool(name="spsum", bufs=1, space="PSUM") as sp,
        tc.tile_pool(name="opsum", bufs=2, space="PSUM") as op_,
        tc.tile_pool(name="oppsum", bufs=2, space="PSUM") as opp,
    ):
        def out_proj(qc, alt_act=False):
            # out-projection for q chunk qc (emitted one chunk late so the
            # PE never fences on the normalize chain); in the drain tail the
            # PSUM->SBUF copies alternate DVE/ACT so both engines drain it
            for ms in range(4):
                sl = slice(qc * 512 + ms * P, qc * 512 + (ms + 1) * P)
                for dc in range(4):
                    pso = opp.tile([P, 512], f32, tag="op", name="pso")
                    for kc in range(2):
                        nc.tensor.matmul(
                            pso, OT[:, kc, sl], wo[:, kc, dc * 512 : (dc + 1) * 512],
                            start=(kc == 0), stop=(kc == 1),
                        )
                    ov = ovp.tile([P, 512], bf, tag="ov", name="ov")
                    if alt_act and dc % 2 == 0:
                        nc.scalar.activation(ov[:], pso[:], Copy)
                    else:
                        nc.vector.tensor_copy(ov[:], pso[:])
                    nc.sync.dma_start(io["out"][sl, dc * 512 : (dc + 1) * 512], ov[:])

        for qc in range(4):
            qs = slice(qc * 512, (qc + 1) * 512)
            nkb = 4 * qc + 4
            for pair in range(2):
                Q = QT[pair]
                po = [op_.tile([HD + 1, 512], f32, tag="o", name="po")
                      for _ in range(2)]

                def score_exp_grp(g):
                    # two key blocks per group: one exp instruction covers
                    # [128, 4, 512] when no diagonal sub-block is involved,
                    # halving exp count + standalone producer waits.
                    # causal trim: only query columns >= the block's first
                    # key are computed; the 128-wide diagonal strip is
                    # masked with the in-block triangle
                    ps4 = sp.tile([P, 2, 2, 512], f32, tag="s", name="ps4")
                    for s in range(2):
                        kb = 2 * g + s
                        c0 = max(kb - 4 * qc, 0) * P
                        kbs = slice(kb * P, (kb + 1) * P)
                        nc.tensor.matmul(ps4[:, s, 0, c0:], KT[0:HD, kbs],
                                         Q[0:HD, qs][:, c0:], start=True,
                                         stop=True, tile_position=(0, 0))
                        nc.tensor.matmul(ps4[:, s, 1, c0:], KT[HD:P, kbs],
                                         Q[HD:P, qs][:, c0:], start=True,
                                         stop=True, tile_position=(HD, 0))
                    if 2 * g + 1 >= 4 * qc:  # group touches the diagonal
                        for s in range(2):
                            kb = 2 * g + s
                            o = kb - 4 * qc
                            c0 = max(o, 0) * P
                            es2 = ering[:, kb, :, :]
                            nc.scalar.activation(es2[:, :, c0:],
                                                 ps4[:, s, :, c0:], Exp,
                                                 scale=SCALE)
                            if o >= 0:
                                nc.vector.tensor_mul(es2[:, :, c0 : c0 + P],
                                                     es2[:, :, c0 : c0 + P],
                                                     masktri[:])
                    else:
                        es4 = ering[:, 2 * g : 2 * g + 2, :, :]
                        nc.scalar.activation(es4[:], ps4[:], Exp, scale=SCALE)

                def pv_grp(g):
                    for s in range(2):
                        kb = 2 * g + s
                        c0 = max(kb - 4 * qc, 0) * P
                        st = (kb == 0)
                        sp_ = (kb == nkb - 1)
                        es2 = ering[:, kb, :, :]
                        for j in range(2):
                            nc.tensor.matmul(po[j][:, c0:], V[:, kb, :],
                                             es2[:, j, c0:], start=st,
                                             stop=sp_)

                ngrp = nkb // 2
                score_exp_grp(0)
                for g in range(1, ngrp):
                    pv_grp(g - 1)
                    score_exp_grp(g)
                pv_grp(ngrp - 1)

                # normalize: row HD of po holds the softmax denominators
                for j in range(2):
                    den = rcp.tile([1, 512], f32, tag="den", name="den")
                    nc.vector.tensor_copy(den, po[j][HD : HD + 1, :])
                    rec = rcp.tile([1, 512], f32, tag="rec", name="rec")
                    nc.vector.reciprocal_approx_fast(rec, den)
                    bcs = bcsp.tile([HD, 512], f32, tag="bcs", name="bcs")
                    nc.gpsimd.partition_broadcast(bcs, rec)
                    if j == 0:
                        nc.vector.tensor_mul(OT[0:HD, pair, qs],
                                             po[j][0:HD, :], bcs)
                    else:
                        stg = stgp.tile([HD, 512], bf, tag="stg", name="stg")
                        nc.vector.tensor_mul(stg, po[j][0:HD, :], bcs)
                        nc.sync.dma_start(OT[HD:P, pair, qs], stg[:])

            if qc > 0:
                out_proj(qc - 1)
        out_proj(3, alt_act=True)


def _prep_core_inputs(i, x, cos, sin, g_q, g_k, Wq, Wk, Wv, Wo):
    c0 = i * 4 * HD
    k0 = i * HD

    def b(a):
        return np.ascontiguousarray(a.astype(BF16))

    x2d = x.reshape(S, D)
    # xt4[p, sc, kc, j] = x[sc*512+j, kc*128+p]
    xt4 = b(x2d.T.reshape(16, P, 4, 512).transpose(1, 2, 0, 3))
    wqa = b(Wq[:, c0 : c0 + P].reshape(16, P, P).transpose(1, 0, 2))
    wqb = b(Wq[:, c0 + P : c0 + 2 * P].reshape(16, P, P).transpose(1, 0, 2))
    wk = b(Wk[:, k0 : k0 + HD].reshape(16, P, HD).transpose(1, 0, 2))
    wv = b(Wv[:, k0 : k0 + HD].reshape(16, P, HD).transpose(1, 0, 2))
    wo = b(Wo[c0 : c0 + 2 * P, :].reshape(2, P, D).transpose(1, 0, 2))
    cosT = cos.T.astype(np.float32)  # [32, S]
    sinT = sin.T.astype(np.float32)
    cos4 = b(np.tile(cosT, (4, 1)))
    sin4s = b(np.concatenate([-sinT, sinT, -sinT, sinT], axis=0))
    gq2 = np.tile(g_q, 2)[:, None].astype(np.float32)
    gk = g_k[:, None].astype(np.float32)
    tri = np.triu(np.ones((P, P), dtype=np.float32))  # [k within blk, q within blk]
    masktri = b(np.stack([tri, tri], axis=1))  # [128, 2, 128]
    ones65 = np.zeros((P, HD + 1), dtype=np.float32)
    ones65[:HD, 0] = 1.0
    ones65[HD:, HD] = 1.0
    r64 = np.roll(np.eye(HD, dtype=np.float32), 32, axis=0)
    rot2 = np.zeros((P, P), dtype=np.float32)
    rot2[:HD, :HD] = r64
    rot2[HD:, HD:] = r64
    return {
        "xt4": xt4,
        "wqa": wqa, "wqb": wqb, "wk": wk, "wv": wv, "wo": wo,
        "cos4": cos4, "sin4s": sin4s,
        "gq2": gq2, "gk": gk, "masktri": masktri,
        "ones65": b(ones65),
        "rot2": b(rot2),
    }


def kernel(x, cos, sin, g_q, g_k, Wq, Wk, Wv, Wo):
    global LAST_RESULTS
    from concourse.bass_utils import run_bass_kernel_spmd

    if "nc" not in _CACHE:
        _CACHE["nc"] = _build_nc()
    nc = _CACHE["nc"]

    args = [np.asarray(a, dtype=np.float32) for a in
            (x, cos, sin, g_q, g_k, Wq, Wk, Wv, Wo)]
    in_maps = [_prep_core_inputs(i, *args) for i in range(N_CORES)]
    trace = bool(os.environ.get("BASS_TRACE"))
    res = run_bass_kernel_spmd(nc, in_maps, list(range(N_CORES)), trace=trace)
    LAST_RESULTS = res
    out = np.zeros((S, D), dtype=np.float32)
    for r in res.results:
        out += np.asarray(r["out"], dtype=np.float32)
    return out.reshape(1, S, D)


# revision 71
# speedup vs baseline: 1.1445x; 1.1445x over previous
"""GQA attention kernel for 8 trn2 NeuronCores (tensor-parallel over heads).

Problem: B=1, S=2048, D=2048, NQ=32 q heads, NKV=8 kv heads, HD=64.
Core i handles q heads 4i..4i+3 and kv head i; out = sum of per-core partials.

v2: all matmuls in bf16 (fp32 runs at 1/4 rate on the PE), x^T prepared on
host (kills 256 on-device PE transposes), paired-head score matmuls issued
to disjoint PE row groups (K=64 each -> concurrent), exp batched over
[128, 2x512] PSUM groups, bf16 partial outputs summed on host.

Layout (all transposed, zero on-device transposes):
  xT  [128, 4(sc), 16(kc), 512] bf16   host-pretransposed activations
  Q^T [128=2 heads x 64, S] per head pair, K^T [128, S] (dup for row pairing)
  V   [S, 16, 64+1] bf16 + ones column (softmax sums come free from PV)
  S^T block pair = KT.T @ QT (two row-group matmuls) -> one exp on ACT
  PV: V_ext.T @ expS^T ; row 64 = softmax denominators
  out-proj: lhsT = O^T directly, bf16 partial written to DRAM
RMSNorm over head dim (= partitions) via ones-selector matmuls on PE.
"""

import os
import sys

sys.path.insert(0, "/opt/trn_rl_repo")

import numpy as np
import ml_dtypes

BF16 = ml_dtypes.bfloat16

S = 2048
D = 2048
HD = 64
NQ = 32
NKV = 8
P = 128
EPS = 1e-6
SCALE = 0.125  # 1/sqrt(HD)
N_CORES = 8

_CACHE = {}
LAST_RESULTS = None


def _build_nc():
    import concourse.bass as bass
    import concourse.tile as tile
    from concourse import bacc, mybir

    f32 = mybir.dt.float32
    bf = mybir.dt.bfloat16
    nc = bacc.Bacc("TRN2", target_bir_lowering=False, debug=False)

    def dram_in(name, shape, dt):
        return nc.dram_tensor(name, list(shape), dt, kind="ExternalInput").ap()

    io = {
        "xt4": dram_in("xt4", (P, 4, 16, 512), bf),
        "wqa": dram_in("wqa", (P, 16, P), bf),
        "wqb": dram_in("wqb", (P, 16, P), bf),
        "wk": dram_in("wk", (P, 16, HD), bf),
        "wv": dram_in("wv", (P, 16, HD), bf),
        "wo": dram_in("wo", (P, 2, D), bf),
        "cos4": dram_in("cos4", (P, S), bf),
        "sin4s": dram_in("sin4s", (P, S), bf),
        "gq2": dram_in("gq2", (P, 1), f32),
        "gk": dram_in("gk", (HD, 1), f32),
        "masktri": dram_in("masktri", (P, 2, P), bf),
        "ones65": dram_in("ones65", (P, HD + 1), bf),
        "rot2": dram_in("rot2", (P, P), bf),
        "out": nc.dram_tensor("out", [S, D], bf, kind="ExternalOutput").ap(),
    }

    from contextlib import ExitStack

    with tile.TileContext(nc) as tc, ExitStack() as ctx:
        _emit(ctx, tc, io, bass, mybir)
    nc.compile()
    return nc


def _emit(ctx, tc, io, bass, mybir):
    nc = tc.nc
    f32 = mybir.dt.float32
    bf = mybir.dt.bfloat16
    Exp = mybir.ActivationFunctionType.Exp
    Sqrt = mybir.ActivationFunctionType.Sqrt
    Square = mybir.ActivationFunctionType.Square
    Copy = mybir.ActivationFunctionType.Copy
    mult = mybir.AluOpType.mult
    add = mybir.AluOpType.add

    cpool = ctx.enter_context(tc.tile_pool(name="consts", bufs=1))
    pers = ctx.enter_context(tc.tile_pool(name="persist", bufs=1))

    # ---- inputs: first seq chunk + first-needed weights lead the DMA queue
    xip = ctx.enter_context(tc.tile_pool(name="xin", bufs=3))
    xs_t = [None] * 4

    def load_xs(sc):
        xs_t[sc] = xip.tile([P, 16, 512], bf, tag="xs", name="xs")
        nc.sync.dma_start(xs_t[sc][:], io["xt4"][:, sc, :, :])

    def cload(name, shape, dt):
        t = cpool.tile(list(shape), dt, tag=name, name=name)
        nc.sync.dma_start(t[:], io[name][:])
        return t

    load_xs(0)
    wqa = cload("wqa", (P, 16, P), bf)
    wk = cload("wk", (P, 16, HD), bf)
    wv = cload("wv", (P, 16, HD), bf)
    wqb = cload("wqb", (P, 16, P), bf)
    gq2 = cload("gq2", (P, 1), f32)
    gk = cload("gk", (HD, 1), f32)
    cos4 = cload("cos4", (P, S), bf)
    sin4s = cload("sin4s", (P, S), bf)
    rot2 = cload("rot2", (P, P), bf)
    ones65 = cload("ones65", (P, HD + 1), bf)
    load_xs(1)
    masktri = cload("masktri", (P, 2, P), bf)
    wo = cload("wo", (P, 2, D), bf)

    # ---- persistent activations ----
    QT = [pers.tile([P, S], bf, tag=f"qt{t}", name=f"QT{t}") for t in range(2)]
    KT = pers.tile([P, S], bf, tag="kt")  # rows 64-127 = duplicate of 0-63
    V = pers.tile([P, 16, HD + 1], bf, tag="v")  # [seq128, kblock, hd+ones]
    OT = pers.tile([P, 2, S], bf, tag="ot")  # attn out transposed

    nc.vector.memset(V[:, :, HD : HD + 1], 1.0)
    epsc = pers.tile([P, 1], f32, tag="epsc")
    nc.vector.memset(epsc[:], EPS)
    jnk = pers.tile([P, P], bf, tag="jnk")
    nc.vector.memset(jnk[:], 0.0)
    # one exp-output slot per key block: fresh region per kb, so the ACT
    # queue carries no buffer-reuse WAR waits inside an attention chunk
    ering = pers.tile([P, 16, 2, 512], bf, tag="ering")

    # ============ Phase 1+2: projections + RMSNorm + RoPE ============
    with (
        tc.tile_pool(name="sq", bufs=2) as sqp,
        tc.tile_pool(name="stdv", bufs=2) as stdp,
        tc.tile_pool(name="rstd", bufs=2) as rsp,
        tc.tile_pool(name="bcast", bufs=2) as bcp,
        tc.tile_pool(name="tnorm", bufs=2) as tnp,
        tc.tile_pool(name="tcos", bufs=2) as tcp,
        tc.tile_pool(name="tsin", bufs=2) as t1p,
        tc.tile_pool(name="ppsum", bufs=3, space="PSUM") as pp,
        tc.tile_pool(name="sspsum", bufs=1, space="PSUM") as ssp,
        tc.tile_pool(name="swpsum", bufs=2, space="PSUM") as swp,
        tc.tile_pool(name="vpsum", bufs=2, space="PSUM") as vp,
    ):
        # warm-up matmuls (no DMA dependency) so the PE clock is already
        # un-throttled when the first projection lands
        for _ in range(96):
            jp = pp.tile([P, 512], f32, tag="p", name="jp")
            nc.tensor.matmul(jp[:, 0:HD], jnk[:], jnk[:, 0:HD], start=True,
                             stop=True)
        for sc in range(4):
            cs = slice(sc * 512, (sc + 1) * 512)
            if sc < 2:
                load_xs(sc + 2)
            xs = xs_t[sc]

            def proj_norm_rope(lhsT_w, m, g, sel, nh, dst):
                # projection into PSUM
                ps = pp.tile([P, 512], f32, tag="p", name="ps")[:m]
                for kc in range(16):
                    nc.tensor.matmul(
                        ps, lhsT_w[:, kc, :], xs[:, kc, :],
                        start=(kc == 0), stop=(kc == 15),
                    )
                # rms stats: sq = ps^2 (bf16); head sums land at partitions
                # 0 and 64 (selector cols 0/64) so all slices are p0/p64
                sq = sqp.tile([P, 512], bf, tag="sq", name="sq")[:m]
                nc.scalar.activation(sq, ps, Square)
                nss = HD + 1 if nh == 2 else 1
                ssps = ssp.tile([HD + 1, 512], f32, tag="ss", name="ssps")[:nss]
                nc.tensor.matmul(ssps, sel[:m, :nss], sq, start=True, stop=True)
                std = stdp.tile([HD + 1, 512], f32, tag="std", name="std")[:nss]
                nc.scalar.activation(std, ssps, Sqrt, bias=epsc[:nss],
                                     scale=1.0 / HD)
                # gpsimd broadcast can only write partition-0-based tiles;
                # the upper head group goes via a staging tile + SBUF DMA
                # approx_fast and partition_broadcast both need partition-0
                # sources on HW; relocate the p64 row via a DVE copy first
                bc = bcp.tile([P, 512], f32, tag="bc", name="bc")[:m]
                for h in range(nh):
                    rstd = rsp.tile([1, 512], f32, tag=f"rstd{h}",
                                    name=f"rstd{h}")
                    if h == 0:
                        nc.vector.reciprocal_approx_fast(rstd, std[0:1, :])
                        nc.gpsimd.partition_broadcast(bc[0:HD, :], rstd)
                    else:
                        stdc = rsp.tile([1, 512], f32, tag="stdc", name="stdc")
                        nc.vector.tensor_copy(stdc, std[HD : HD + 1, :])
                        nc.vector.reciprocal_approx_fast(rstd, stdc)
                        bch = bcp.tile([HD, 512], f32, tag="bch", name="bch")
                        nc.gpsimd.partition_broadcast(bch, rstd)
                        nc.sync.dma_start(bc[HD:P, :], bch[:])
                # normalize: tn = (ps * g) * bc   (bf16 out)
                tn = tnp.tile([P, 512], bf, tag="tn", name="tn")[:m]
                nc.vector.scalar_tensor_tensor(tn, ps, g, bc, mult, mult)
                # rope: dst = tn*cos + swap(tn)*sin
                tmpc = tcp.tile([P, 512], bf, tag="tc", name="tmpc")[:m]
                nc.vector.tensor_mul(tmpc, tn, cos4[:m, cs])
                sw = swp.tile([P, 512], f32, tag="sw", name="sw")[:m]
                nc.tensor.matmul(sw, rot2[:m, :m], tn, start=True, stop=True)
                t1 = t1p.tile([P, 512], bf, tag="t1", name="t1")[:m]
                nc.vector.tensor_mul(t1, sw, sin4s[:m, cs])
                nc.vector.tensor_add(dst, t1, tmpc)

            proj_norm_rope(wqa, P, gq2[:, :], ones65, 2, QT[0][:, cs])
            proj_norm_rope(wqb, P, gq2[:, :], ones65, 2, QT[1][:, cs])
            proj_norm_rope(wk, HD, gk[:, :], ones65, 1, KT[0:HD, cs])
            # duplicate normed+roped K into partitions 64-127 (row pairing)
            nc.sync.dma_start(KT[HD:P, cs], KT[0:HD, cs])
            # V in normal layout: lhsT = x^T slice, rhs = wv
            for ms in range(4):
                pv = vp.tile([P, HD], f32, tag="v", name="pv")
                for kc in range(16):
                    nc.tensor.matmul(
                        pv[:], xs[:, kc, ms * P : (ms + 1) * P], wv[:, kc, :],
                        start=(kc == 0), stop=(kc == 15),
                    )
                nc.vector.tensor_copy(V[:, sc * 4 + ms, 0:HD], pv[:])

    # ============ Phase 3: attention + out-projection ============
    with (
        tc.tile_pool(name="exps", bufs=4) as ep,
        tc.tile_pool(name="recs", bufs=2) as rcp,
        tc.tile_pool(name="bcs", bufs=2) as bcsp,
        tc.tile_pool(name="stg", bufs=2) as stgp,
        tc.tile_pool(name="ov", bufs=2) as ovp,
        tc.tile_pool(name="spsum", bufs=2, space="PSUM") as sp,
        tc.tile_pool(name="opsum", bufs=2, space="PSUM") as op_,
        tc.tile_pool(name="oppsum", bufs=2, space="PSUM") as opp,
    ):
        def out_proj(qc, alt_act=False):
            # out-projection for q chunk qc (emitted one chunk late so the
            # PE never fences on the normalize chain); in the drain tail the
            # PSUM->SBUF copies alternate DVE/ACT so both engines drain it
            for ms in range(4):
                sl = slice(qc * 512 + ms * P, qc * 512 + (ms + 1) * P)
                for dc in range(4):
                    pso = opp.tile([P, 512], f32, tag="op", name="pso")
                    for kc in range(2):
                        nc.tensor.matmul(
                            pso, OT[:, kc, sl], wo[:, kc, dc * 512 : (dc + 1) * 512],
                            start=(kc == 0), stop=(kc == 1),
                        )
                    ov = ovp.tile([P, 512], bf, tag="ov", name="ov")
                    if alt_act and dc % 2 == 0:
                        nc.scalar.activation(ov[:], pso[:], Copy)
                    else:
                        nc.vector.tensor_copy(ov[:], pso[:])
                    nc.sync.dma_start(io["out"][sl, dc * 512 : (dc + 1) * 512], ov[:])

        for qc in range(4):
            qs = slice(qc * 512, (qc + 1) * 512)
            nkb = 4 * qc + 4
            for pair in range(2):
                Q = QT[pair]
                po = [op_.tile([HD + 1, 512], f32, tag="o", name="po")
                      for _ in range(2)]

                def score_exp(kb):
                    # causal trim: only query columns >= the block's first
                    # key are computed; the 128-wide diagonal strip is
                    # masked with the in-block triangle
                    o = kb - 4 * qc
                    c0 = max(o, 0) * P
                    ps2 = sp.tile([P, 2, 512], f32, tag="s", name="ps2")
                    kbs = slice(kb * P, (kb + 1) * P)
                    nc.tensor.matmul(ps2[:, 0, c0:], KT[0:HD, kbs],
                                     Q[0:HD, qs][:, c0:], start=True,
                                     stop=True, tile_position=(0, 0))
                    nc.tensor.matmul(ps2[:, 1, c0:], KT[HD:P, kbs],
                                     Q[HD:P, qs][:, c0:], start=True,
                                     stop=True, tile_position=(HD, 0))
                    es2 = ering[:, kb, :, :]
                    nc.scalar.activation(es2[:, :, c0:], ps2[:, :, c0:], Exp,
                                         scale=SCALE)
                    if o >= 0:
                        nc.vector.tensor_mul(es2[:, :, c0 : c0 + P],
                                             es2[:, :, c0 : c0 + P],
                                             masktri[:])
                    return es2

                def pv_acc(kb, es2):
                    c0 = max(kb - 4 * qc, 0) * P
                    st = (kb == 0)
                    sp_ = (kb == nkb - 1)
                    for j in range(2):
                        nc.tensor.matmul(po[j][:, c0:], V[:, kb, :],
                                         es2[:, j, c0:], start=st, stop=sp_)

                prev = score_exp(0)
                for kb in range(1, nkb):
                    cur = score_exp(kb)
                    pv_acc(kb - 1, prev)
                    prev = cur
                pv_acc(nkb - 1, prev)

                # normalize: row HD of po holds the softmax denominators
                for j in range(2):
                    den = rcp.tile([1, 512], f32, tag="den", name="den")
                    nc.vector.tensor_copy(den, po[j][HD : HD + 1, :])
                    rec = rcp.tile([1, 512], f32, tag="rec", name="rec")
                    nc.vector.reciprocal_approx_fast(rec, den)
                    bcs = bcsp.tile([HD, 512], f32, tag="bcs", name="bcs")
                    nc.gpsimd.partition_broadcast(bcs, rec)
                    if j == 0:
                        nc.vector.tensor_mul(OT[0:HD, pair, qs],
                                             po[j][0:HD, :], bcs)
                    else:
                        stg = stgp.tile([HD, 512], bf, tag="stg", name="stg")
                        nc.vector.tensor_mul(stg, po[j][0:HD, :], bcs)
                        nc.sync.dma_start(OT[HD:P, pair, qs], stg[:])

            if qc > 0:
                out_proj(qc - 1)
        out_proj(3, alt_act=True)


def _prep_core_inputs(i, x, cos, sin, g_q, g_k, Wq, Wk, Wv, Wo):
    c0 = i * 4 * HD
    k0 = i * HD

    def b(a):
        return np.ascontiguousarray(a.astype(BF16))

    x2d = x.reshape(S, D)
    # xt4[p, sc, kc, j] = x[sc*512+j, kc*128+p]
    xt4 = b(x2d.T.reshape(16, P, 4, 512).transpose(1, 2, 0, 3))
    wqa = b(Wq[:, c0 : c0 + P].reshape(16, P, P).transpose(1, 0, 2))
    wqb = b(Wq[:, c0 + P : c0 + 2 * P].reshape(16, P, P).transpose(1, 0, 2))
    wk = b(Wk[:, k0 : k0 + HD].reshape(16, P, HD).transpose(1, 0, 2))
    wv = b(Wv[:, k0 : k0 + HD].reshape(16, P, HD).transpose(1, 0, 2))
    wo = b(Wo[c0 : c0 + 2 * P, :].reshape(2, P, D).transpose(1, 0, 2))
    cosT = cos.T.astype(np.float32)  # [32, S]
    sinT = sin.T.astype(np.float32)
    cos4 = b(np.tile(cosT, (4, 1)))
    sin4s = b(np.concatenate([-sinT, sinT, -sinT, sinT], axis=0))
    gq2 = np.tile(g_q, 2)[:, None].astype(np.float32)
    gk = g_k[:, None].astype(np.float32)
    tri = np.triu(np.ones((P, P), dtype=np.float32))  # [k within blk, q within blk]
    masktri = b(np.stack([tri, tri], axis=1))  # [128, 2, 128]
    ones65 = np.zeros((P, HD + 1), dtype=np.float32)
    ones65[:HD, 0] = 1.0
    ones65[HD:, HD] = 1.0
    r64 = np.roll(np.eye(HD, dtype=np.float32), 32, axis=0)
    rot2 = np.zeros((P, P), dtype=np.float32)
    rot2[:HD, :HD] = r64
    rot2[HD:, HD:] = r64
    return {
        "xt4": xt4,
        "wqa": wqa, "wqb": wqb, "wk": wk, "wv": wv, "wo": wo,
        "cos4": cos4, "sin4s": sin4s,
        "gq2": gq2, "gk": gk, "masktri": masktri,
        "ones65": b(ones65),
        "rot2": b(rot2),
    }


def kernel(x, cos, sin, g_q, g_k, Wq, Wk, Wv, Wo):
    global LAST_RESULTS
    from concourse.bass_utils import run_bass_kernel_spmd

    if "nc" not in _CACHE:
        _CACHE["nc"] = _build_nc()
    nc = _CACHE["nc"]

    args = [np.asarray(a, dtype=np.float32) for a in
            (x, cos, sin, g_q, g_k, Wq, Wk, Wv, Wo)]
    in_maps = [_prep_core_inputs(i, *args) for i in range(N_CORES)]
    trace = bool(os.environ.get("BASS_TRACE"))
    res = run_bass_kernel_spmd(nc, in_maps, list(range(N_CORES)), trace=trace)
    LAST_RESULTS = res
    out = np.zeros((S, D), dtype=np.float32)
    for r in res.results:
        out += np.asarray(r["out"], dtype=np.float32)
    return out.reshape(1, S, D)


# revision 72
# speedup vs baseline: 1.1543x; 1.0086x over previous
"""GQA attention kernel for 8 trn2 NeuronCores (tensor-parallel over heads).

Problem: B=1, S=2048, D=2048, NQ=32 q heads, NKV=8 kv heads, HD=64.
Core i handles q heads 4i..4i+3 and kv head i; out = sum of per-core partials.

v2: all matmuls in bf16 (fp32 runs at 1/4 rate on the PE), x^T prepared on
host (kills 256 on-device PE transposes), paired-head score matmuls issued
to disjoint PE row groups (K=64 each -> concurrent), exp batched over
[128, 2x512] PSUM groups, bf16 partial outputs summed on host.

Layout (all transposed, zero on-device transposes):
  xT  [128, 4(sc), 16(kc), 512] bf16   host-pretransposed activations
  Q^T [128=2 heads x 64, S] per head pair, K^T [128, S] (dup for row pairing)
  V   [S, 16, 64+1] bf16 + ones column (softmax sums come free from PV)
  S^T block pair = KT.T @ QT (two row-group matmuls) -> one exp on ACT
  PV: V_ext.T @ expS^T ; row 64 = softmax denominators
  out-proj: lhsT = O^T directly, bf16 partial written to DRAM
RMSNorm over head dim (= partitions) via ones-selector matmuls on PE.
"""

import os
import sys

sys.path.insert(0, "/opt/trn_rl_repo")

import numpy as np
import ml_dtypes

BF16 = ml_dtypes.bfloat16

S = 2048
D = 2048
HD = 64
NQ = 32
NKV = 8
P = 128
EPS = 1e-6
SCALE = 0.125  # 1/sqrt(HD)
N_CORES = 8

_CACHE = {}
LAST_RESULTS = None


def _build_nc():
    import concourse.bass as bass
    import concourse.tile as tile
    from concourse import bacc, mybir

    f32 = mybir.dt.float32
    bf = mybir.dt.bfloat16
    nc = bacc.Bacc("TRN2", target_bir_lowering=False, debug=False)

    def dram_in(name, shape, dt):
        return nc.dram_tensor(name, list(shape), dt, kind="ExternalInput").ap()

    io = {
        "xt4": dram_in("xt4", (P, 4, 16, 512), bf),
        "wqa": dram_in("wqa", (P, 16, P), bf),
        "wqb": dram_in("wqb", (P, 16, P), bf),
        "wk": dram_in("wk", (P, 16, HD), bf),
        "wv": dram_in("wv", (P, 16, HD), bf),
        "wo": dram_in("wo", (P, 2, D), bf),
        "cos4": dram_in("cos4", (P, S), bf),
        "sin4s": dram_in("sin4s", (P, S), bf),
        "gq2": dram_in("gq2", (P, 1), f32),
        "gk": dram_in("gk", (HD, 1), f32),
        "masktri": dram_in("masktri", (P, 2, P), bf),
        "ones65": dram_in("ones65", (P, HD + 1), bf),
        "rot2": dram_in("rot2", (P, P), bf),
        "out": nc.dram_tensor("out", [S, D], bf, kind="ExternalOutput").ap(),
    }

    from contextlib import ExitStack

    with tile.TileContext(nc) as tc, ExitStack() as ctx:
        _emit(ctx, tc, io, bass, mybir)
    nc.compile()
    return nc


def _emit(ctx, tc, io, bass, mybir):
    nc = tc.nc
    f32 = mybir.dt.float32
    bf = mybir.dt.bfloat16
    Exp = mybir.ActivationFunctionType.Exp
    Sqrt = mybir.ActivationFunctionType.Sqrt
    Square = mybir.ActivationFunctionType.Square
    Copy = mybir.ActivationFunctionType.Copy
    mult = mybir.AluOpType.mult
    add = mybir.AluOpType.add

    cpool = ctx.enter_context(tc.tile_pool(name="consts", bufs=1))
    pers = ctx.enter_context(tc.tile_pool(name="persist", bufs=1))

    # ---- inputs: first seq chunk + first-needed weights lead the DMA queue
    xip = ctx.enter_context(tc.tile_pool(name="xin", bufs=3))
    xs_t = [None] * 4

    def load_xs(sc):
        xs_t[sc] = xip.tile([P, 16, 512], bf, tag="xs", name="xs")
        nc.sync.dma_start(xs_t[sc][:], io["xt4"][:, sc, :, :])

    def cload(name, shape, dt):
        t = cpool.tile(list(shape), dt, tag=name, name=name)
        nc.sync.dma_start(t[:], io[name][:])
        return t

    load_xs(0)
    wqa = cload("wqa", (P, 16, P), bf)
    wk = cload("wk", (P, 16, HD), bf)
    wv = cload("wv", (P, 16, HD), bf)
    wqb = cload("wqb", (P, 16, P), bf)
    gq2 = cload("gq2", (P, 1), f32)
    gk = cload("gk", (HD, 1), f32)
    cos4 = cload("cos4", (P, S), bf)
    sin4s = cload("sin4s", (P, S), bf)
    rot2 = cload("rot2", (P, P), bf)
    ones65 = cload("ones65", (P, HD + 1), bf)
    load_xs(1)
    masktri = cload("masktri", (P, 2, P), bf)
    wo = cload("wo", (P, 2, D), bf)

    # ---- persistent activations ----
    QT = [pers.tile([P, S], bf, tag=f"qt{t}", name=f"QT{t}") for t in range(2)]
    KT = pers.tile([P, S], bf, tag="kt")  # rows 64-127 = duplicate of 0-63
    V = pers.tile([P, 16, HD + 1], bf, tag="v")  # [seq128, kblock, hd+ones]
    OT = pers.tile([P, 2, S], bf, tag="ot")  # attn out transposed

    nc.vector.memset(V[:, :, HD : HD + 1], 1.0)
    epsc = pers.tile([P, 1], f32, tag="epsc")
    nc.vector.memset(epsc[:], EPS)
    jnk = pers.tile([P, P], bf, tag="jnk")
    nc.vector.memset(jnk[:], 0.0)
    # one exp-output slot per key block: fresh region per kb, so the ACT
    # queue carries no buffer-reuse WAR waits inside an attention chunk
    ering = pers.tile([P, 16, 2, 512], bf, tag="ering")

    # ============ Phase 1+2: projections + RMSNorm + RoPE ============
    with (
        tc.tile_pool(name="sq", bufs=2) as sqp,
        tc.tile_pool(name="stdv", bufs=2) as stdp,
        tc.tile_pool(name="rstd", bufs=2) as rsp,
        tc.tile_pool(name="bcast", bufs=2) as bcp,
        tc.tile_pool(name="tnorm", bufs=2) as tnp,
        tc.tile_pool(name="tcos", bufs=2) as tcp,
        tc.tile_pool(name="tsin", bufs=2) as t1p,
        tc.tile_pool(name="ppsum", bufs=3, space="PSUM") as pp,
        tc.tile_pool(name="sspsum", bufs=1, space="PSUM") as ssp,
        tc.tile_pool(name="swpsum", bufs=2, space="PSUM") as swp,
        tc.tile_pool(name="vpsum", bufs=2, space="PSUM") as vp,
    ):
        # warm-up matmuls (no DMA dependency) so the PE clock is already
        # un-throttled when the first projection lands
        for _ in range(96):
            jp = pp.tile([P, 512], f32, tag="p", name="jp")
            nc.tensor.matmul(jp[:, 0:HD], jnk[:], jnk[:, 0:HD], start=True,
                             stop=True)
        for sc in range(4):
            cs = slice(sc * 512, (sc + 1) * 512)
            if sc < 2:
                load_xs(sc + 2)
            xs = xs_t[sc]

            def proj_norm_rope(lhsT_w, m, g, sel, nh, dst):
                # projection into PSUM
                ps = pp.tile([P, 512], f32, tag="p", name="ps")[:m]
                for kc in range(16):
                    nc.tensor.matmul(
                        ps, lhsT_w[:, kc, :], xs[:, kc, :],
                        start=(kc == 0), stop=(kc == 15),
                    )
                # rms stats: sq = ps^2 (bf16); head sums land at partitions
                # 0 and 64 (selector cols 0/64) so all slices are p0/p64
                sq = sqp.tile([P, 512], bf, tag="sq", name="sq")[:m]
                nc.scalar.activation(sq, ps, Square)
                nss = HD + 1 if nh == 2 else 1
                ssps = ssp.tile([HD + 1, 512], f32, tag="ss", name="ssps")[:nss]
                nc.tensor.matmul(ssps, sel[:m, :nss], sq, start=True, stop=True)
                std = stdp.tile([HD + 1, 512], f32, tag="std", name="std")[:nss]
                nc.scalar.activation(std, ssps, Sqrt, bias=epsc[:nss],
                                     scale=1.0 / HD)
                # gpsimd broadcast can only write partition-0-based tiles;
                # the upper head group goes via a staging tile + SBUF DMA
                # approx_fast and partition_broadcast both need partition-0
                # sources on HW; relocate the p64 row via a DVE copy first
                bc = bcp.tile([P, 512], f32, tag="bc", name="bc")[:m]
                for h in range(nh):
                    rstd = rsp.tile([1, 512], f32, tag=f"rstd{h}",
                                    name=f"rstd{h}")
                    if h == 0:
                        nc.vector.reciprocal_approx_fast(rstd, std[0:1, :])
                        nc.gpsimd.partition_broadcast(bc[0:HD, :], rstd)
                    else:
                        stdc = rsp.tile([1, 512], f32, tag="stdc", name="stdc")
                        nc.vector.tensor_copy(stdc, std[HD : HD + 1, :])
                        nc.vector.reciprocal_approx_fast(rstd, stdc)
                        bch = bcp.tile([HD, 512], f32, tag="bch", name="bch")
                        nc.gpsimd.partition_broadcast(bch, rstd)
                        nc.sync.dma_start(bc[HD:P, :], bch[:])
                # normalize: tn = (ps * g) * bc   (bf16 out)
                tn = tnp.tile([P, 512], bf, tag="tn", name="tn")[:m]
                nc.vector.scalar_tensor_tensor(tn, ps, g, bc, mult, mult)
                # rope: dst = tn*cos + swap(tn)*sin
                tmpc = tcp.tile([P, 512], bf, tag="tc", name="tmpc")[:m]
                nc.vector.tensor_mul(tmpc, tn, cos4[:m, cs])
                sw = swp.tile([P, 512], f32, tag="sw", name="sw")[:m]
                nc.tensor.matmul(sw, rot2[:m, :m], tn, start=True, stop=True)
                t1 = t1p.tile([P, 512], bf, tag="t1", name="t1")[:m]
                nc.vector.tensor_mul(t1, sw, sin4s[:m, cs])
                nc.vector.tensor_add(dst, t1, tmpc)

            proj_norm_rope(wqa, P, gq2[:, :], ones65, 2, QT[0][:, cs])
            proj_norm_rope(wqb, P, gq2[:, :], ones65, 2, QT[1][:, cs])
            proj_norm_rope(wk, HD, gk[:, :], ones65, 1, KT[0:HD, cs])
            # duplicate normed+roped K into partitions 64-127 (row pairing)
            nc.sync.dma_start(KT[HD:P, cs], KT[0:HD, cs])
            # V in normal layout: lhsT = x^T slice, rhs = wv
            for ms in range(4):
                pv = vp.tile([P, HD], f32, tag="v", name="pv")
                for kc in range(16):
                    nc.tensor.matmul(
                        pv[:], xs[:, kc, ms * P : (ms + 1) * P], wv[:, kc, :],
                        start=(kc == 0), stop=(kc == 15),
                    )
                nc.vector.tensor_copy(V[:, sc * 4 + ms, 0:HD], pv[:])

    # ============ Phase 3: attention + out-projection ============
    with (
        tc.tile_pool(name="exps", bufs=4) as ep,
        tc.tile_pool(name="recs", bufs=2) as rcp,
        tc.tile_pool(name="bcs", bufs=2) as bcsp,
        tc.tile_pool(name="stg", bufs=2) as stgp,
        tc.tile_pool(name="ov", bufs=2) as ovp,
        tc.tile_pool(name="spsum", bufs=2, space="PSUM") as sp,
        tc.tile_pool(name="opsum", bufs=2, space="PSUM") as op_,
        tc.tile_pool(name="oppsum", bufs=2, space="PSUM") as opp,
    ):
        def out_proj(qc, alt_act=False):
            # out-projection for q chunk qc (emitted one chunk late so the
            # PE never fences on the normalize chain); in the drain tail the
            # PSUM->SBUF copies alternate DVE/ACT so both engines drain it
            for ms in range(4):
                sl = slice(qc * 512 + ms * P, qc * 512 + (ms + 1) * P)
                for dc in range(4):
                    pso = opp.tile([P, 512], f32, tag="op", name="pso")
                    for kc in range(2):
                        nc.tensor.matmul(
                            pso, OT[:, kc, sl], wo[:, kc, dc * 512 : (dc + 1) * 512],
                            start=(kc == 0), stop=(kc == 1),
                        )
                    ov = ovp.tile([P, 512], bf, tag="ov", name="ov")
                    if alt_act and dc % 2 == 0:
                        nc.scalar.activation(ov[:], pso[:], Copy)
                    else:
                        nc.vector.tensor_copy(ov[:], pso[:])
                    nc.sync.dma_start(io["out"][sl, dc * 512 : (dc + 1) * 512], ov[:])

        for qc in range(4):
            qs = slice(qc * 512, (qc + 1) * 512)
            nkb = 4 * qc + 4
            for pair in range(2):
                Q = QT[pair]
                po = [op_.tile([HD + 1, 512], f32, tag="o", name="po")
                      for _ in range(2)]

                def score_exp(kb):
                    # causal trim: only query columns >= the block's first
                    # key are computed; the 128-wide diagonal strip is
                    # masked with the in-block triangle
                    o = kb - 4 * qc
                    c0 = max(o, 0) * P
                    ps2 = sp.tile([P, 2, 512], f32, tag="s", name="ps2")
                    kbs = slice(kb * P, (kb + 1) * P)
                    # HAM feeder: junk matmul into a region the scores
                    # overwrite (c0=0) or the exp never reads (c0>=128)
                    nc.tensor.matmul(ps2[:, 0, 0:HD], jnk[:], jnk[:, 0:HD],
                                     start=True, stop=True)
                    nc.tensor.matmul(ps2[:, 0, c0:], KT[0:HD, kbs],
                                     Q[0:HD, qs][:, c0:], start=True,
                                     stop=True, tile_position=(0, 0))
                    nc.tensor.matmul(ps2[:, 1, c0:], KT[HD:P, kbs],
                                     Q[HD:P, qs][:, c0:], start=True,
                                     stop=True, tile_position=(HD, 0))
                    es2 = ering[:, kb, :, :]
                    nc.scalar.activation(es2[:, :, c0:], ps2[:, :, c0:], Exp,
                                         scale=SCALE)
                    if o >= 0:
                        nc.vector.tensor_mul(es2[:, :, c0 : c0 + P],
                                             es2[:, :, c0 : c0 + P],
                                             masktri[:])
                    return es2

                def pv_acc(kb, es2):
                    c0 = max(kb - 4 * qc, 0) * P
                    st = (kb == 0)
                    sp_ = (kb == nkb - 1)
                    for j in range(2):
                        nc.tensor.matmul(po[j][:, c0:], V[:, kb, :],
                                         es2[:, j, c0:], start=st, stop=sp_)

                prev = score_exp(0)
                for kb in range(1, nkb):
                    cur = score_exp(kb)
                    pv_acc(kb - 1, prev)
                    prev = cur
                pv_acc(nkb - 1, prev)

                # normalize: row HD of po holds the softmax denominators
                for j in range(2):
                    den = rcp.tile([1, 512], f32, tag="den", name="den")
                    nc.vector.tensor_copy(den, po[j][HD : HD + 1, :])
                    rec = rcp.tile([1, 512], f32, tag="rec", name="rec")
                    nc.vector.reciprocal_approx_fast(rec, den)
                    bcs = bcsp.tile([HD, 512], f32, tag="bcs", name="bcs")
                    nc.gpsimd.partition_broadcast(bcs, rec)
                    if j == 0:
                        nc.vector.tensor_mul(OT[0:HD, pair, qs],
                                             po[j][0:HD, :], bcs)
                    else:
                        stg = stgp.tile([HD, 512], bf, tag="stg", name="stg")
                        nc.vector.tensor_mul(stg, po[j][0:HD, :], bcs)
                        nc.sync.dma_start(OT[HD:P, pair, qs], stg[:])

            if qc > 0:
                out_proj(qc - 1)
        out_proj(3, alt_act=True)


def _prep_core_inputs(i, x, cos, sin, g_q, g_k, Wq, Wk, Wv, Wo):
    c0 = i * 4 * HD
    k0 = i * HD

    def b(a):
        return np.ascontiguousarray(a.astype(BF16))

    x2d = x.reshape(S, D)
    # xt4[p, sc, kc, j] = x[sc*512+j, kc*128+p]
    xt4 = b(x2d.T.reshape(16, P, 4, 512).transpose(1, 2, 0, 3))
    wqa = b(Wq[:, c0 : c0 + P].reshape(16, P, P).transpose(1, 0, 2))
    wqb = b(Wq[:, c0 + P : c0 + 2 * P].reshape(16, P, P).transpose(1, 0, 2))
    wk = b(Wk[:, k0 : k0 + HD].reshape(16, P, HD).transpose(1, 0, 2))
    wv = b(Wv[:, k0 : k0 + HD].reshape(16, P, HD).transpose(1, 0, 2))
    wo = b(Wo[c0 : c0 + 2 * P, :].reshape(2, P, D).transpose(1, 0, 2))
    cosT = cos.T.astype(np.float32)  # [32, S]
    sinT = sin.T.astype(np.float32)
    cos4 = b(np.tile(cosT, (4, 1)))
    sin4s = b(np.concatenate([-sinT, sinT, -sinT, sinT], axis=0))
    gq2 = np.tile(g_q, 2)[:, None].astype(np.float32)
    gk = g_k[:, None].astype(np.float32)
    tri = np.triu(np.ones((P, P), dtype=np.float32))  # [k within blk, q within blk]
    masktri = b(np.stack([tri, tri], axis=1))  # [128, 2, 128]
    ones65 = np.zeros((P, HD + 1), dtype=np.float32)
    ones65[:HD, 0] = 1.0
    ones65[HD:, HD] = 1.0
    r64 = np.roll(np.eye(HD, dtype=np.float32), 32, axis=0)
    rot2 = np.zeros((P, P), dtype=np.float32)
    rot2[:HD, :HD] = r64
    rot2[HD:, HD:] = r64
    return {
        "xt4": xt4,
        "wqa": wqa, "wqb": wqb, "wk": wk, "wv": wv, "wo": wo,
        "cos4": cos4, "sin4s": sin4s,
        "gq2": gq2, "gk": gk, "masktri": masktri,
        "ones65": b(ones65),
        "rot2": b(rot2),
    }


def kernel(x, cos, sin, g_q, g_k, Wq, Wk, Wv, Wo):
    global LAST_RESULTS
    from concourse.bass_utils import run_bass_kernel_spmd

    if "nc" not in _CACHE:
        _CACHE["nc"] = _build_nc()
    nc = _CACHE["nc"]

    args = [np.asarray(a, dtype=np.float32) for a in
            (x, cos, sin, g_q, g_k, Wq, Wk, Wv, Wo)]
    in_maps = [_prep_core_inputs(i, *args) for i in range(N_CORES)]
    trace = bool(os.environ.get("BASS_TRACE"))
    res = run_bass_kernel_spmd(nc, in_maps, list(range(N_CORES)), trace=trace)
    LAST_RESULTS = res
    out = np.zeros((S, D), dtype=np.float32)
    for r in res.results:
        out += np.asarray(r["out"], dtype=np.float32)
    return out.reshape(1, S, D)


# revision 73
# speedup vs baseline: 1.1919x; 1.0325x over previous
"""GQA attention kernel for 8 trn2 NeuronCores (tensor-parallel over heads).

Problem: B=1, S=2048, D=2048, NQ=32 q heads, NKV=8 kv heads, HD=64.
Core i handles q heads 4i..4i+3 and kv head i; out = sum of per-core partials.

v2: all matmuls in bf16 (fp32 runs at 1/4 rate on the PE), x^T prepared on
host (kills 256 on-device PE transposes), paired-head score matmuls issued
to disjoint PE row groups (K=64 each -> concurrent), exp batched over
[128, 2x512] PSUM groups, bf16 partial outputs summed on host.

Layout (all transposed, zero on-device transposes):
  xT  [128, 4(sc), 16(kc), 512] bf16   host-pretransposed activations
  Q^T [128=2 heads x 64, S] per head pair, K^T [128, S] (dup for row pairing)
  V   [S, 16, 64+1] bf16 + ones column (softmax sums come free from PV)
  S^T block pair = KT.T @ QT (two row-group matmuls) -> one exp on ACT
  PV: V_ext.T @ expS^T ; row 64 = softmax denominators
  out-proj: lhsT = O^T directly, bf16 partial written to DRAM
RMSNorm over head dim (= partitions) via ones-selector matmuls on PE.
"""

import os
import sys

sys.path.insert(0, "/opt/trn_rl_repo")

import numpy as np
import ml_dtypes

BF16 = ml_dtypes.bfloat16

S = 2048
D = 2048
HD = 64
NQ = 32
NKV = 8
P = 128
EPS = 1e-6
SCALE = 0.125  # 1/sqrt(HD)
N_CORES = 8

_CACHE = {}
LAST_RESULTS = None


def _build_nc():
    import concourse.bass as bass
    import concourse.tile as tile
    from concourse import bacc, mybir

    f32 = mybir.dt.float32
    bf = mybir.dt.bfloat16
    nc = bacc.Bacc("TRN2", target_bir_lowering=False, debug=False)

    def dram_in(name, shape, dt):
        return nc.dram_tensor(name, list(shape), dt, kind="ExternalInput").ap()

    io = {
        "xt4": dram_in("xt4", (P, 4, 16, 512), bf),
        "wqa": dram_in("wqa", (P, 16, P), bf),
        "wqb": dram_in("wqb", (P, 16, P), bf),
        "wk": dram_in("wk", (P, 16, HD), bf),
        "wv": dram_in("wv", (P, 16, HD), bf),
        "wo": dram_in("wo", (P, 2, D), bf),
        "cos4": dram_in("cos4", (P, S), bf),
        "sin4s": dram_in("sin4s", (P, S), bf),
        "gq2": dram_in("gq2", (P, 1), f32),
        "gk": dram_in("gk", (HD, 1), f32),
        "masktri": dram_in("masktri", (P, 2, P), bf),
        "ones65": dram_in("ones65", (P, HD + 1), bf),
        "rot2": dram_in("rot2", (P, P), bf),
        "out": nc.dram_tensor("out", [S, D], bf, kind="ExternalOutput").ap(),
    }

    from contextlib import ExitStack

    with tile.TileContext(nc) as tc, ExitStack() as ctx:
        _emit(ctx, tc, io, bass, mybir)
    nc.compile()
    return nc


def _emit(ctx, tc, io, bass, mybir):
    nc = tc.nc
    f32 = mybir.dt.float32
    bf = mybir.dt.bfloat16
    Exp = mybir.ActivationFunctionType.Exp
    Sqrt = mybir.ActivationFunctionType.Sqrt
    Square = mybir.ActivationFunctionType.Square
    Copy = mybir.ActivationFunctionType.Copy
    mult = mybir.AluOpType.mult
    add = mybir.AluOpType.add

    cpool = ctx.enter_context(tc.tile_pool(name="consts", bufs=1))
    pers = ctx.enter_context(tc.tile_pool(name="persist", bufs=1))

    # ---- inputs: first seq chunk + first-needed weights lead the DMA queue
    xip = ctx.enter_context(tc.tile_pool(name="xin", bufs=3))
    xs_t = [None] * 4

    def load_xs(sc):
        xs_t[sc] = xip.tile([P, 16, 512], bf, tag="xs", name="xs")
        nc.sync.dma_start(xs_t[sc][:], io["xt4"][:, sc, :, :])

    def cload(name, shape, dt):
        t = cpool.tile(list(shape), dt, tag=name, name=name)
        nc.sync.dma_start(t[:], io[name][:])
        return t

    load_xs(0)
    wqa = cload("wqa", (P, 16, P), bf)
    wk = cload("wk", (P, 16, HD), bf)
    wv = cload("wv", (P, 16, HD), bf)
    wqb = cload("wqb", (P, 16, P), bf)
    gq2 = cload("gq2", (P, 1), f32)
    gk = cload("gk", (HD, 1), f32)
    cos4 = cload("cos4", (P, S), bf)
    sin4s = cload("sin4s", (P, S), bf)
    rot2 = cload("rot2", (P, P), bf)
    ones65 = cload("ones65", (P, HD + 1), bf)
    load_xs(1)
    masktri = cload("masktri", (P, 2, P), bf)
    wo = cload("wo", (P, 2, D), bf)

    # ---- persistent activations ----
    QT = [pers.tile([P, S], bf, tag=f"qt{t}", name=f"QT{t}") for t in range(2)]
    KT = pers.tile([P, S], bf, tag="kt")  # rows 64-127 = duplicate of 0-63
    V = pers.tile([P, 16, HD + 1], bf, tag="v")  # [seq128, kblock, hd+ones]
    OT = pers.tile([P, 2, S], bf, tag="ot")  # attn out transposed

    nc.vector.memset(V[:, :, HD : HD + 1], 1.0)
    epsc = pers.tile([P, 1], f32, tag="epsc")
    nc.vector.memset(epsc[:], EPS)
    jnk = pers.tile([P, P], bf, tag="jnk")
    nc.vector.memset(jnk[:], 0.0)
    # one exp-output slot per key block: fresh region per kb, so the ACT
    # queue carries no buffer-reuse WAR waits inside an attention chunk
    ering = pers.tile([P, 16, 2, 512], bf, tag="ering")

    # ============ Phase 1+2: projections + RMSNorm + RoPE ============
    with (
        tc.tile_pool(name="sq", bufs=2) as sqp,
        tc.tile_pool(name="stdv", bufs=2) as stdp,
        tc.tile_pool(name="rstd", bufs=2) as rsp,
        tc.tile_pool(name="bcast", bufs=2) as bcp,
        tc.tile_pool(name="tnorm", bufs=2) as tnp,
        tc.tile_pool(name="tcos", bufs=2) as tcp,
        tc.tile_pool(name="tsin", bufs=2) as t1p,
        tc.tile_pool(name="ppsum", bufs=3, space="PSUM") as pp,
        tc.tile_pool(name="sspsum", bufs=1, space="PSUM") as ssp,
        tc.tile_pool(name="swpsum", bufs=2, space="PSUM") as swp,
        tc.tile_pool(name="vpsum", bufs=2, space="PSUM") as vp,
    ):
        # warm-up matmuls (no DMA dependency) so the PE clock is already
        # un-throttled when the first projection lands
        for _ in range(96):
            jp = pp.tile([P, 512], f32, tag="p", name="jp")
            nc.tensor.matmul(jp[:, 0:HD], jnk[:], jnk[:, 0:HD], start=True,
                             stop=True)
        for sc in range(4):
            cs = slice(sc * 512, (sc + 1) * 512)
            if sc < 2:
                load_xs(sc + 2)
            xs = xs_t[sc]

            def proj_norm_rope(lhsT_w, m, g, sel, nh, dst):
                # projection into PSUM
                ps = pp.tile([P, 512], f32, tag="p", name="ps")[:m]
                for kc in range(16):
                    nc.tensor.matmul(
                        ps, lhsT_w[:, kc, :], xs[:, kc, :],
                        start=(kc == 0), stop=(kc == 15),
                    )
                # rms stats: sq = ps^2 (bf16); head sums land at partitions
                # 0 and 64 (selector cols 0/64) so all slices are p0/p64
                sq = sqp.tile([P, 512], bf, tag="sq", name="sq")[:m]
                nc.scalar.activation(sq, ps, Square)
                nss = HD + 1 if nh == 2 else 1
                ssps = ssp.tile([HD + 1, 512], f32, tag="ss", name="ssps")[:nss]
                nc.tensor.matmul(ssps, sel[:m, :nss], sq, start=True, stop=True)
                std = stdp.tile([HD + 1, 512], f32, tag="std", name="std")[:nss]
                nc.scalar.activation(std, ssps, Sqrt, bias=epsc[:nss],
                                     scale=1.0 / HD)
                # gpsimd broadcast can only write partition-0-based tiles;
                # the upper head group goes via a staging tile + SBUF DMA
                # approx_fast and partition_broadcast both need partition-0
                # sources on HW; relocate the p64 row via a DVE copy first
                bc = bcp.tile([P, 512], f32, tag="bc", name="bc")[:m]
                for h in range(nh):
                    rstd = rsp.tile([1, 512], f32, tag=f"rstd{h}",
                                    name=f"rstd{h}")
                    if h == 0:
                        nc.vector.reciprocal_approx_fast(rstd, std[0:1, :])
                        nc.gpsimd.partition_broadcast(bc[0:HD, :], rstd)
                    else:
                        stdc = rsp.tile([1, 512], f32, tag="stdc", name="stdc")
                        nc.vector.tensor_copy(stdc, std[HD : HD + 1, :])
                        nc.vector.reciprocal_approx_fast(rstd, stdc)
                        bch = bcp.tile([HD, 512], f32, tag="bch", name="bch")
                        nc.gpsimd.partition_broadcast(bch, rstd)
                        nc.sync.dma_start(bc[HD:P, :], bch[:])
                # normalize: tn = (ps * g) * bc   (bf16 out)
                tn = tnp.tile([P, 512], bf, tag="tn", name="tn")[:m]
                nc.vector.scalar_tensor_tensor(tn, ps, g, bc, mult, mult)
                # rope: dst = tn*cos + swap(tn)*sin
                tmpc = tcp.tile([P, 512], bf, tag="tc", name="tmpc")[:m]
                nc.vector.tensor_mul(tmpc, tn, cos4[:m, cs])
                sw = swp.tile([P, 512], f32, tag="sw", name="sw")[:m]
                nc.tensor.matmul(sw, rot2[:m, :m], tn, start=True, stop=True)
                t1 = t1p.tile([P, 512], bf, tag="t1", name="t1")[:m]
                nc.vector.tensor_mul(t1, sw, sin4s[:m, cs])
                nc.vector.tensor_add(dst, t1, tmpc)

            proj_norm_rope(wqa, P, gq2[:, :], ones65, 2, QT[0][:, cs])
            proj_norm_rope(wqb, P, gq2[:, :], ones65, 2, QT[1][:, cs])
            proj_norm_rope(wk, HD, gk[:, :], ones65, 1, KT[0:HD, cs])
            # duplicate normed+roped K into partitions 64-127 (row pairing)
            nc.sync.dma_start(KT[HD:P, cs], KT[0:HD, cs])
            # V in normal layout: lhsT = x^T slice, rhs = wv
            for ms in range(4):
                pv = vp.tile([P, HD], f32, tag="v", name="pv")
                for kc in range(16):
                    nc.tensor.matmul(
                        pv[:], xs[:, kc, ms * P : (ms + 1) * P], wv[:, kc, :],
                        start=(kc == 0), stop=(kc == 15),
                    )
                nc.vector.tensor_copy(V[:, sc * 4 + ms, 0:HD], pv[:])

    # ============ Phase 3: attention + out-projection ============
    with (
        tc.tile_pool(name="exps", bufs=4) as ep,
        tc.tile_pool(name="recs", bufs=2) as rcp,
        tc.tile_pool(name="bcs", bufs=2) as bcsp,
        tc.tile_pool(name="stg", bufs=2) as stgp,
        tc.tile_pool(name="ov", bufs=3) as ovp,
        tc.tile_pool(name="spsum", bufs=2, space="PSUM") as sp,
        tc.tile_pool(name="opsum", bufs=2, space="PSUM") as op_,
        tc.tile_pool(name="oppsum", bufs=2, space="PSUM") as opp,
    ):
        def out_proj(qc, alt_act=False):
            # out-projection for q chunk qc (emitted one chunk late so the
            # PE never fences on the normalize chain); in the drain tail the
            # PSUM->SBUF copies alternate DVE/ACT so both engines drain it
            for ms in range(4):
                sl = slice(qc * 512 + ms * P, qc * 512 + (ms + 1) * P)
                for dc in range(4):
                    pso = opp.tile([P, 512], f32, tag="op", name="pso")
                    for kc in range(2):
                        nc.tensor.matmul(
                            pso, OT[:, kc, sl], wo[:, kc, dc * 512 : (dc + 1) * 512],
                            start=(kc == 0), stop=(kc == 1),
                        )
                    ov = ovp.tile([P, 512], bf, tag="ov", name="ov")
                    if alt_act and dc % 2 == 0:
                        nc.scalar.activation(ov[:], pso[:], Copy)
                    else:
                        nc.vector.tensor_copy(ov[:], pso[:])
                    nc.sync.dma_start(io["out"][sl, dc * 512 : (dc + 1) * 512], ov[:])

        for qc in range(4):
            qs = slice(qc * 512, (qc + 1) * 512)
            nkb = 4 * qc + 4
            for pair in range(2):
                Q = QT[pair]
                po = [op_.tile([HD + 1, 512], f32, tag="o", name="po")
                      for _ in range(2)]

                def score_exp(kb):
                    # causal trim: only query columns >= the block's first
                    # key are computed; the 128-wide diagonal strip is
                    # masked with the in-block triangle
                    o = kb - 4 * qc
                    c0 = max(o, 0) * P
                    ps2 = sp.tile([P, 2, 512], f32, tag="s", name="ps2")
                    kbs = slice(kb * P, (kb + 1) * P)
                    # HAM feeder: junk matmul into a region the scores
                    # overwrite (c0=0) or the exp never reads (c0>=128)
                    nc.tensor.matmul(ps2[:, 0, 0:HD], jnk[:], jnk[:, 0:HD],
                                     start=True, stop=True)
                    nc.tensor.matmul(ps2[:, 0, c0:], KT[0:HD, kbs],
                                     Q[0:HD, qs][:, c0:], start=True,
                                     stop=True, tile_position=(0, 0))
                    nc.tensor.matmul(ps2[:, 1, c0:], KT[HD:P, kbs],
                                     Q[HD:P, qs][:, c0:], start=True,
                                     stop=True, tile_position=(HD, 0))
                    es2 = ering[:, kb, :, :]
                    nc.scalar.activation(es2[:, :, c0:], ps2[:, :, c0:], Exp,
                                         scale=SCALE)
                    if o >= 0:
                        nc.vector.tensor_mul(es2[:, :, c0 : c0 + P],
                                             es2[:, :, c0 : c0 + P],
                                             masktri[:])
                    return es2

                def pv_acc(kb, es2):
                    c0 = max(kb - 4 * qc, 0) * P
                    st = (kb == 0)
                    sp_ = (kb == nkb - 1)
                    for j in range(2):
                        nc.tensor.matmul(po[j][:, c0:], V[:, kb, :],
                                         es2[:, j, c0:], start=st, stop=sp_)

                prev = score_exp(0)
                for kb in range(1, nkb):
                    cur = score_exp(kb)
                    pv_acc(kb - 1, prev)
                    prev = cur
                pv_acc(nkb - 1, prev)

                # normalize: row HD of po holds the softmax denominators.
                # issue both heads' den/rec/broadcast up front so the two
                # gpsimd broadcasts overlap; junk matmuls hold the PE clock
                # through the fence
                bcs2 = []
                for j in range(2):
                    den = rcp.tile([1, 512], f32, tag=f"den{j}",
                                   name=f"den{j}")
                    nc.vector.tensor_copy(den, po[j][HD : HD + 1, :])
                    rec = rcp.tile([1, 512], f32, tag=f"rec{j}",
                                   name=f"rec{j}")
                    nc.vector.reciprocal_approx_fast(rec, den)
                    bcs = bcsp.tile([HD, 512], f32, tag=f"bcs{j}",
                                    name=f"bcs{j}")
                    nc.gpsimd.partition_broadcast(bcs, rec)
                    bcs2.append(bcs)
                jfence = sp.tile([P, 2, 512], f32, tag="s", name="jfence")
                for _ in range(10):
                    nc.tensor.matmul(jfence[:, 0, 0:HD], jnk[:],
                                     jnk[:, 0:HD], start=True, stop=True)
                for j in range(2):
                    if j == 0:
                        nc.vector.tensor_mul(OT[0:HD, pair, qs],
                                             po[j][0:HD, :], bcs2[j])
                    else:
                        stg = stgp.tile([HD, 512], bf, tag="stg", name="stg")
                        nc.vector.tensor_mul(stg, po[j][0:HD, :], bcs2[j])
                        nc.sync.dma_start(OT[HD:P, pair, qs], stg[:])

            if qc > 0:
                out_proj(qc - 1)
        out_proj(3, alt_act=True)


def _prep_core_inputs(i, x, cos, sin, g_q, g_k, Wq, Wk, Wv, Wo):
    c0 = i * 4 * HD
    k0 = i * HD

    def b(a):
        return np.ascontiguousarray(a.astype(BF16))

    x2d = x.reshape(S, D)
    # xt4[p, sc, kc, j] = x[sc*512+j, kc*128+p]
    xt4 = b(x2d.T.reshape(16, P, 4, 512).transpose(1, 2, 0, 3))
    wqa = b(Wq[:, c0 : c0 + P].reshape(16, P, P).transpose(1, 0, 2))
    wqb = b(Wq[:, c0 + P : c0 + 2 * P].reshape(16, P, P).transpose(1, 0, 2))
    wk = b(Wk[:, k0 : k0 + HD].reshape(16, P, HD).transpose(1, 0, 2))
    wv = b(Wv[:, k0 : k0 + HD].reshape(16, P, HD).transpose(1, 0, 2))
    wo = b(Wo[c0 : c0 + 2 * P, :].reshape(2, P, D).transpose(1, 0, 2))
    cosT = cos.T.astype(np.float32)  # [32, S]
    sinT = sin.T.astype(np.float32)
    cos4 = b(np.tile(cosT, (4, 1)))
    sin4s = b(np.concatenate([-sinT, sinT, -sinT, sinT], axis=0))
    gq2 = np.tile(g_q, 2)[:, None].astype(np.float32)
    gk = g_k[:, None].astype(np.float32)
    tri = np.triu(np.ones((P, P), dtype=np.float32))  # [k within blk, q within blk]
    masktri = b(np.stack([tri, tri], axis=1))  # [128, 2, 128]
    ones65 = np.zeros((P, HD + 1), dtype=np.float32)
    ones65[:HD, 0] = 1.0
    ones65[HD:, HD] = 1.0
    r64 = np.roll(np.eye(HD, dtype=np.float32), 32, axis=0)
    rot2 = np.zeros((P, P), dtype=np.float32)
    rot2[:HD, :HD] = r64
    rot2[HD:, HD:] = r64
    return {
        "xt4": xt4,
        "wqa": wqa, "wqb": wqb, "wk": wk, "wv": wv, "wo": wo,
        "cos4": cos4, "sin4s": sin4s,
        "gq2": gq2, "gk": gk, "masktri": masktri,
        "ones65": b(ones65),
        "rot2": b(rot2),
    }


def kernel(x, cos, sin, g_q, g_k, Wq, Wk, Wv, Wo):
    global LAST_RESULTS
    from concourse.bass_utils import run_bass_kernel_spmd

    if "nc" not in _CACHE:
        _CACHE["nc"] = _build_nc()
    nc = _CACHE["nc"]

    args = [np.asarray(a, dtype=np.float32) for a in
            (x, cos, sin, g_q, g_k, Wq, Wk, Wv, Wo)]
    in_maps = [_prep_core_inputs(i, *args) for i in range(N_CORES)]
    trace = bool(os.environ.get("BASS_TRACE"))
    res = run_bass_kernel_spmd(nc, in_maps, list(range(N_CORES)), trace=trace)
    LAST_RESULTS = res
    out = np.zeros((S, D), dtype=np.float32)
    for r in res.results:
        out += np.asarray(r["out"], dtype=np.float32)
    return out.reshape(1, S, D)
